# revision 4
# baseline (speedup 1.0000x reference)
"""CountHistogram Trainium2 kernel (v5: base-8 packed septet counting).

Reference computation:
    bins = trunc((simmat + 1.00001) / 2 * 29)            (values >= 0)
    w[b,q,d] = (dtoks[b,d] != -1) & (qtoks[b,q] != -1)
    hist[b,c,q,n] = sum_d w[b,q,d] * (bins[b,c,q,d] == n)

Strategy (pure data-parallel over 8 NeuronCores, B=128 sharded 16/core).
Data is host-pre-transposed so d lives on partitions:
    simt[b, p, c*128+r] = sim[b, r, c*128+p]   (p: d%128, c: d//128, r: row)

Per batch b:
1) PE penalty matmul (K=19) broadcasts per-element penalties into PSUM:
       penbc[p, j] = dpenA[d] + qpen[r] + r_hi + r_lo
   dpenA = 14 - 2000*dmask (rank-16 over chunk one-hots), qpen = -1000*qmask,
   r_hi + r_lo = bf16-split of 1.00001*14.5 - 0.5 - 14 (exact residual).
2) One DVE scalar_tensor_tensor: u = int16(simt*14.5 + penbc).  RNE with the
   folded -0.5 gives floor(); valid elements get the reference bin in [0,29],
   masked elements land near -2000/-3000 (below every window).
3) Counting, 7 bins per pass ("septets"): for s in 0..3
       tc  = clip(u, 7s-1, 7s+7)                (DVE tensor_scalar, 4x)
       phi = exp(ln8 * (tc - 7s))               (ACT; exact powers of 8 in bf16)
       PE matmul with a groups-of-7-partitions lhsT accumulates phi into PSUM.
   Each PSUM cell sums <= 7 elements, so the base-8 digits of the sum are the
   per-(group, chunk, row) counts of the 7 bins (+ a below fraction and an
   above-count in the top field).  Max sum 7*8^7 < 2^24 -> exact fp32.
4) Bin 29 is a plain is_equal mask counted through an all-ones lhsT column;
   bin 28 = (septet3 above-count) - c29.
5) DVE copies the [77, 2048] count PSUM to SBUF; DMA to HBM; host unpacks
   base-8 fields and folds the 19 groups x 16 chunks.
"""

import math
import sys

import numpy as np

sys.path.insert(0, "/opt/trn_rl_repo")

NBINS = 30
B, C, Q, D = 128, 4, 32, 2048
NCORES = 8
BS = B // NCORES  # batches per core
ROWS = C * Q  # 128
NCHUNK = D // 128  # 16
NSEP = 4  # septet streams (bins 0..27)
NGRP = 19  # 18 groups of 7 partitions + 1 group of 2
MROWS = NSEP * NGRP + 1  # 77 psum rows (last = bin-29 singles)
LN8 = math.log(8.0)

_CACHE = {}
LAST_RESULTS = None


def _build():
    import concourse.bacc as bacc  # noqa
    import concourse.bass as bass  # noqa
    import concourse.mybir as mybir
    import concourse.tile as tile

    A = mybir.AluOpType
    dt = mybir.dt
    AF = mybir.ActivationFunctionType

    nc = bacc.Bacc("TRN2", target_bir_lowering=False, debug=False, num_devices=NCORES)

    simt = nc.dram_tensor("simt", [BS, 128, D], dt.float32, kind="ExternalInput")
    plhs = nc.dram_tensor("plhs", [BS, 19, 128], dt.bfloat16, kind="ExternalInput")
    prhs = nc.dram_tensor("prhs", [BS, 19, D], dt.bfloat16, kind="ExternalInput")
    lgt = nc.dram_tensor("lgt", [128, 5 * MROWS], dt.bfloat16, kind="ExternalInput")
    hps = nc.dram_tensor("hps", [BS, MROWS, D], dt.float32, kind="ExternalOutput")

    with tile.TileContext(nc) as tc_:
        with (
            tc_.tile_pool(name="const", bufs=1) as cpool,
            tc_.tile_pool(name="sim", bufs=2) as simpool,
            tc_.tile_pool(name="work", bufs=2) as wpool,
            tc_.tile_pool(name="clip", bufs=2) as tpool,
            tc_.tile_pool(name="phi", bufs=3) as fpool,
            tc_.tile_pool(name="small", bufs=2) as spool,
            tc_.tile_pool(name="evac", bufs=2) as epool,
            tc_.tile_pool(name="psumA", bufs=1, space="PSUM") as psumA,
            tc_.tile_pool(name="psumB", bufs=1, space="PSUM") as psumB,
        ):
            lg = cpool.tile([128, 5 * MROWS], dt.bfloat16)
            nc.sync.dma_start(lg[:], lgt[:, :])
            biases = cpool.tile([128, NSEP], dt.float32)
            for s in range(NSEP):
                nc.vector.memset(biases[:, s : s + 1], float(-7 * s * LN8))

            for b in range(BS):
                st = simpool.tile([128, D], dt.float32, tag="sim")
                nc.sync.dma_start(st[:], simt[b, :, :])
                pl = spool.tile([19, 128], dt.bfloat16, tag="plhs")
                nc.sync.dma_start(pl[:], plhs[b, :, :])
                pr = spool.tile([19, D], dt.bfloat16, tag="prhs")
                nc.sync.dma_start(pr[:], prhs[b, :, :])

                penbc = psumA.tile([128, D], dt.float32, tag="penbc")
                for j in range(0, D, 512):
                    nc.tensor.matmul(
                        penbc[:, j : j + 512],
                        pl[:],
                        pr[:, j : j + 512],
                        start=True,
                        stop=True,
                    )

                ut = wpool.tile([128, D], dt.int16, tag="ut")
                nc.vector.scalar_tensor_tensor(
                    ut[:], st[:], 14.5, penbc[:], A.mult, A.add
                )

                counts = psumB.tile([MROWS, D], dt.float32, tag="counts")
                for s in range(NSEP):
                    tcl = tpool.tile([128, D], dt.int16, tag="tc")
                    nc.vector.tensor_scalar(
                        tcl[:], ut[:], float(7 * s - 1), float(7 * s + 7), A.max, A.min
                    )
                    ph = fpool.tile([128, D], dt.bfloat16, tag="ph")
                    nc.scalar.activation(
                        ph[:], tcl[:], AF.Exp, bias=biases[:, s : s + 1], scale=LN8
                    )
                    for j in range(0, D, 512):
                        nc.tensor.matmul(
                            counts[:, j : j + 512],
                            lg[:, MROWS * s : MROWS * (s + 1)],
                            ph[:, j : j + 512],
                            start=(s == 0),
                            stop=False,
                            skip_group_check=True,
                        )

                mk = fpool.tile([128, D], dt.bfloat16, tag="ph")
                nc.vector.tensor_scalar(mk[:], ut[:], 29.0, None, A.is_equal)
                for j in range(0, D, 512):
                    nc.tensor.matmul(
                        counts[:, j : j + 512],
                        lg[:, MROWS * 4 : MROWS * 5],
                        mk[:, j : j + 512],
                        start=False,
                        stop=True,
                        skip_group_check=True,
                    )

                ev = epool.tile([MROWS, D], dt.float32, tag="ev")
                nc.vector.tensor_copy(ev[:], counts[:])
                nc.sync.dma_start(hps[b, :, :], ev[:])

    nc.compile()
    return nc


def _get_nc():
    if "nc" not in _CACHE:
        _CACHE["nc"] = _build()
    return _CACHE["nc"]


def _host_prep(simmat, dtoks, qtoks):
    import ml_dtypes

    bf = ml_dtypes.bfloat16

    # Exact-penalty constants (same derivation as validated v4 kernel).
    c0 = np.float32(np.float32(1.00001) * np.float32(14.5))
    chalf = np.float32(c0 - np.float32(0.5))  # 14.000145...
    r = np.float32(chalf - np.float32(14.0))  # exact residual
    r_hi = np.float32(bf(r))
    r_lo = np.float32(r - r_hi)

    # simt[b, p, c*128+r] = sim[b, r, c*128+p]
    sim_rows = simmat.reshape(B, ROWS, NCHUNK, 128)
    simt = np.ascontiguousarray(sim_rows.transpose(0, 3, 2, 1)).reshape(B, 128, D)

    # pen lhsT [19, 128]: rows 0..15 dpenA per chunk, 16 ones, 17 r_hi, 18 r_lo
    plhs = np.zeros((B, 19, 128), bf)
    dpenA = np.float32(14.0) + np.where(
        dtoks == -1, np.float32(-2000.0), np.float32(0.0)
    )  # [B, D]
    plhs[:, 0:16, :] = dpenA.reshape(B, NCHUNK, 128).astype(bf)
    plhs[:, 16, :] = bf(1.0)
    plhs[:, 17, :] = bf(r_hi)
    plhs[:, 18, :] = bf(r_lo)

    # pen rhs [19, D]: rows 0..15 chunk one-hots, 16 qpen over j=(c,r), 17/18 ones
    prhs = np.zeros((B, 19, D), bf)
    eye = np.zeros((NCHUNK, D), np.float32)
    for k in range(NCHUNK):
        eye[k, k * 128 : (k + 1) * 128] = 1.0
    prhs[:, 0:16, :] = eye.astype(bf)[None]
    qpen_q = np.where(qtoks == -1, np.float32(-1000.0), np.float32(0.0))  # [B, Q]
    qpen_row = np.tile(qpen_q, (1, C))  # [B, ROWS] row r -> q = r % Q
    prhs[:, 16, :] = np.tile(qpen_row, (1, NCHUNK)).astype(bf)
    prhs[:, 17, :] = bf(1.0)
    prhs[:, 18, :] = bf(1.0)

    # count lhsT: 4 septet streams (groups of 7 partitions) + all-ones single
    LS = np.zeros((5, 128, MROWS), np.float32)
    for s in range(NSEP):
        for g in range(NGRP):
            p0 = 7 * g
            p1 = min(p0 + 7, 128)
            LS[s, p0:p1, NGRP * s + g] = 1.0
    LS[4, :, MROWS - 1] = 1.0
    lgt = np.ascontiguousarray(LS.transpose(1, 0, 2)).reshape(128, 5 * MROWS).astype(bf)

    return simt, plhs, prhs, lgt


def kernel(simmat, dlens, dtoks, qtoks):
    global LAST_RESULTS
    from concourse.bass_utils import run_bass_kernel_spmd

    simmat = np.ascontiguousarray(simmat, dtype=np.float32)
    dtoks = np.asarray(dtoks)
    qtoks = np.asarray(qtoks)

    simt, plhs, prhs, lgt = _host_prep(simmat, dtoks, qtoks)

    nc = _get_nc()

    in_maps = []
    for core in range(NCORES):
        lo, hi = core * BS, (core + 1) * BS
        in_maps.append(
            {
                "simt": np.ascontiguousarray(simt[lo:hi]),
                "plhs": np.ascontiguousarray(plhs[lo:hi]),
                "prhs": np.ascontiguousarray(prhs[lo:hi]),
                "lgt": lgt,
            }
        )

    res = run_bass_kernel_spmd(nc, in_maps, core_ids=list(range(NCORES)))
    LAST_RESULTS = res

    full = np.zeros((B, ROWS, NBINS), np.float32)
    for core in range(NCORES):
        lo = core * BS
        hp = res.results[core]["hps"]  # [BS, 77, D] f32
        # septet fields: S8 = n_below + sum_k 8^(k+1) c_k + 8^8 n_above
        S8 = np.rint(
            hp[:, : NSEP * NGRP, :].astype(np.float64) * 8.0
        ).astype(np.int64)
        S8 = S8.reshape(BS, NSEP, NGRP, NCHUNK, 128)  # [b, s, g, c, r]
        for k in range(7):
            fld = (S8 >> (3 * (k + 1))) & 7
            cnt = fld.sum(axis=(2, 3), dtype=np.int64)  # [BS, NSEP, 128]
            for s in range(NSEP):
                full[lo : lo + BS, :, 7 * s + k] = cnt[:, s, :]
        # above-count of septet 3 = c28 + c29
        ca3 = ((S8[:, 3] >> 24) & 7).sum(axis=(1, 2), dtype=np.int64)  # [BS, 128]
        c29 = hp[:, MROWS - 1, :].reshape(BS, NCHUNK, 128).sum(axis=1)
        full[lo : lo + BS, :, 29] = c29
        full[lo : lo + BS, :, 28] = ca3 - c29

    return full.reshape(B, C, Q, NBINS).astype(np.float32)
